# revision 17
# baseline (speedup 1.0000x reference)
"""AxialAttention Trainium2 kernel (8-core SPMD, batch-parallel).

Problem: q,k,v [2,8,4,512,8,64] f32, attention over axis 3 (H=512).
Flattened: 512 independent attentions of [S=512, c=64], shared masks.
Outputs: out (same shape as q) and attn [512, 512, 512] probabilities.

Sharding: 64 (b,h,t) groups x 8 w-batches; core i owns groups [8i, 8i+8).
Host pre-transposes q,k into [g, wpair, (2w,c)=128, S] fp16 tiles and
pre-casts v to fp16, so the device does no q/k/v preprocessing.

Per-core pipeline per batch (w):
  S(psum)[ic] = ident16 @ maskbias16  +  qT16.T @ kT16     (fp16 PE)
  M, denom    = exp(S/8) with accum_out (ACT; mask folded in additively)
  r           = 1/denom                (DVE reciprocal)
  A           = M * head_mask          (DVE tensor_tensor f32)
  attn        = A * r                  (DVE tensor_scalar)  -> DMA out
  a16         = fp16(attn)             (DVE cast)
  a16^T       = PE transposes (fp16) -> psum -> evict fp16 (ACT/DVE)
  out[ic]     = sum_jc a16^T[jc,ic].T @ v16[jc]  (fp16 PE) -> copy -> DMA
"""
import sys
import numpy as np

sys.path.insert(0, "/opt/trn_rl_repo")

import concourse.bacc as bacc
import concourse.tile as tile
from concourse import mybir
from concourse.bass_utils import run_bass_kernel_spmd
from concourse.masks import make_identity

F32 = mybir.dt.float32
F16 = mybir.dt.float16
EXP = mybir.ActivationFunctionType.Exp
COPY = mybir.ActivationFunctionType.Copy
MUL = mybir.AluOpType.mult
BYP = mybir.AluOpType.bypass

N_CORES = 8
G_PER_CORE = 8   # (b,h,t) groups per core
W = 8            # w batches per group
S = 512          # sequence (H axis)
C = 64           # head dim
IC = S // 128    # i-chunks per batch
MASK_NEG = -49152.0  # exactly representable in fp16; exp((S+this)/8) == 0


def build_nc():
    nc = bacc.Bacc("TRN2", target_bir_lowering=False, debug=False,
                   num_devices=N_CORES)

    qT_ext = nc.dram_tensor("qT", [G_PER_CORE, 4, 128, S], F16, kind="ExternalInput")
    kT_ext = nc.dram_tensor("kT", [G_PER_CORE, 4, 128, S], F16, kind="ExternalInput")
    v_ext = nc.dram_tensor("v", [G_PER_CORE, S, W * C], F16, kind="ExternalInput")
    mb_ext = nc.dram_tensor("maskbias", [S, S], F16, kind="ExternalInput")
    hm_ext = nc.dram_tensor("headmask", [S, S], F32, kind="ExternalInput")
    attn_ext = nc.dram_tensor("attn", [G_PER_CORE * W, S, S], F32, kind="ExternalOutput")
    out_ext = nc.dram_tensor("out", [G_PER_CORE, S, W * C], F32, kind="ExternalOutput")

    with tile.TileContext(nc) as tc:
        with (
            tc.tile_pool(name="const", bufs=1) as const,
            tc.tile_pool(name="slab", bufs=12) as slab,
            tc.tile_pool(name="trans", bufs=8) as trans,
            tc.tile_pool(name="soft", bufs=8) as soft,
            tc.tile_pool(name="att", bufs=14) as att,
            tc.tile_pool(name="small", bufs=40) as small,
            tc.tile_pool(name="oslab", bufs=2) as oslab,
            tc.tile_pool(name="ps_s", bufs=4, space="PSUM") as ps_s,
            tc.tile_pool(name="ps_ta", bufs=1, space="PSUM") as ps_ta,
            tc.tile_pool(name="ps_o", bufs=2, space="PSUM") as ps_o,
        ):
            identf = const.tile([128, 128], F32)
            make_identity(nc, identf)
            ident16 = const.tile([128, 128], F16)
            nc.vector.tensor_copy(ident16, identf)

            # masks (shared across all batches), as 4 i-chunk tiles [128, S]
            mb16 = []
            hm_t = []
            for ic in range(IC):
                t = const.tile([128, S], F16, tag=f"mb{ic}")
                nc.sync.dma_start(out=t, in_=mb_ext[ic * 128:(ic + 1) * 128, :])
                mb16.append(t)
                th = const.tile([128, S], F16, tag=f"hm{ic}")
                nc.gpsimd.dma_start(out=th, in_=hm_ext[ic * 128:(ic + 1) * 128, :])
                hm_t.append(th)

            for g in range(G_PER_CORE):
                # ---- load qT/kT (fp16, pre-transposed on host) and v16 ----
                qT, kT, v16 = [], [], []
                for wp in range(4):
                    qt = slab.tile([128, S], F16, tag="qT")
                    nc.sync.dma_start(out=qt, in_=qT_ext[g, wp])
                    qT.append(qt)
                    kt = slab.tile([128, S], F16, tag="kT")
                    nc.sync.dma_start(out=kt, in_=kT_ext[g, wp])
                    kT.append(kt)
                for hc in range(IC):
                    v6 = slab.tile([128, W * C], F16, tag="v16")
                    nc.sync.dma_start(out=v6, in_=v_ext[g, hc * 128:(hc + 1) * 128, :])
                    v16.append(v6)

                o_slab = [oslab.tile([128, W * C], F32, tag=f"os{ic}", name=f"oslab{ic}")
                          for ic in range(IC)]

                for w in range(W):
                    wp, half = w // 2, (w % 2) * 64
                    b = g * W + w

                    sps = []
                    for ic in range(IC):
                        sp = ps_s.tile([128, S], F32, tag="S")
                        nc.tensor.matmul(sp, lhsT=ident16, rhs=mb16[ic],
                                         start=True, stop=False)
                        sps.append(sp)
                    for ic in range(IC):
                        nc.tensor.matmul(
                            sps[ic],
                            lhsT=qT[wp][half:half + 64, ic * 128:(ic + 1) * 128],
                            rhs=kT[wp][half:half + 64, :],
                            start=False, stop=True,
                        )
                    ta_banks = [ps_ta.tile([128, 2 * S], F16, tag=f"ta{i}", name=f"ta{i}")
                                for i in range(2)]
                    m_ts = []
                    den = small.tile([128, IC], F32, tag="den")
                    for ic in range(IC):
                        m_t = soft.tile([128, S], F16, tag="M")
                        nc.scalar.activation(m_t, sps[ic], EXP, scale=0.125,
                                             accum_out=den[:, ic:ic + 1])
                        m_ts.append(m_t)
                    r_t = small.tile([128, IC], F32, tag="r")
                    nc.vector.reciprocal(r_t, den)
                    for ic in range(IC):
                        m_t = m_ts[ic]
                        a_t = soft.tile([128, S], F16, tag="A")
                        nc.vector.tensor_tensor(out=a_t, in0=m_t, in1=hm_t[ic],
                                                op=MUL)
                        a16 = soft.tile([128, S], F16, tag="A16")
                        nc.vector.tensor_scalar(out=a16, in0=a_t,
                                                scalar1=r_t[:, ic:ic + 1],
                                                op0=MUL, scalar2=None, op1=BYP)
                        at_t = att.tile([128, S], F32, tag="attn")
                        if ic % 2 == 0:
                            nc.gpsimd.tensor_copy(at_t, a16)
                        else:
                            nc.vector.tensor_copy(at_t, a16)
                        nc.sync.dma_start(
                            out=attn_ext[b, ic * 128:(ic + 1) * 128, :],
                            in_=at_t,
                        )
                        for jc in range(IC):
                            nc.tensor.transpose(
                                ta_banks[jc // 2][:, (jc % 2) * S + ic * 128:
                                                  (jc % 2) * S + (ic + 1) * 128],
                                a16[:, jc * 128:(jc + 1) * 128],
                                ident16,
                            )
                    atT = []
                    for jc in range(IC):
                        t = trans.tile([128, S], F16, tag=f"atT{jc}", bufs=6)
                        src = ta_banks[jc // 2][:, (jc % 2) * S:(jc % 2 + 1) * S]
                        if jc <= 1:
                            nc.vector.tensor_copy(t, src)
                        else:
                            nc.scalar.activation(t, src, COPY)
                        atT.append(t)

                    for ic in range(IC):
                        op = ps_o.tile([128, C], F32, tag="o")
                        for jc in range(IC):
                            nc.tensor.matmul(
                                op,
                                lhsT=atT[jc][:, ic * 128:(ic + 1) * 128],
                                rhs=v16[jc][:, w * C:(w + 1) * C],
                                start=(jc == 0), stop=(jc == IC - 1),
                            )
                        nc.vector.tensor_copy(
                            o_slab[ic][:, w * C:(w + 1) * C], op)

                for ic in range(IC):
                    nc.sync.dma_start(
                        out=out_ext[g, ic * 128:(ic + 1) * 128, :],
                        in_=o_slab[ic],
                    )

    nc.compile()
    return nc


_NC_CACHE = None


def make_in_maps(q, k, v, attention_mask, head_mask):
    q = np.ascontiguousarray(q, dtype=np.float32)
    k = np.ascontiguousarray(k, dtype=np.float32)
    v = np.ascontiguousarray(v, dtype=np.float32)
    B, H, T, Sdim, Wdim, Cdim = q.shape  # [2,8,4,512,8,64]
    NG = B * H * T                        # 64 groups

    # [NG, S, W, C] -> [NG, wp=4, (wodd,c)=128, S] fp16
    def to_T(x):
        xr = x.reshape(NG, Sdim, 4, 2, Cdim)
        return np.ascontiguousarray(
            xr.transpose(0, 2, 3, 4, 1).reshape(NG, 4, 2 * Cdim, Sdim)
        ).astype(np.float16)

    qT = to_T(q)
    kT = to_T(k)
    v16 = v.reshape(NG, Sdim, Wdim * Cdim).astype(np.float16)

    mask = np.asarray(attention_mask).reshape(Sdim, Sdim)
    maskbias = (((mask == 0).astype(np.float32)) * np.float32(MASK_NEG)).astype(np.float16)
    hm = np.ascontiguousarray(
        np.asarray(head_mask).reshape(Sdim, Sdim), dtype=np.float32)

    in_maps = []
    for c in range(N_CORES):
        sl = slice(c * G_PER_CORE, (c + 1) * G_PER_CORE)
        in_maps.append({
            "qT": np.ascontiguousarray(qT[sl]),
            "kT": np.ascontiguousarray(kT[sl]),
            "v": np.ascontiguousarray(v16[sl]),
            "maskbias": maskbias,
            "headmask": hm,
        })
    return in_maps, (B, H, T, Sdim, Wdim, Cdim)


def kernel(q, k, v, attention_mask, head_mask):
    global _NC_CACHE
    if _NC_CACHE is None:
        _NC_CACHE = build_nc()
    nc = _NC_CACHE

    in_maps, (B, H, T, Sdim, Wdim, Cdim) = make_in_maps(
        q, k, v, attention_mask, head_mask)
    NG = B * H * T

    res = run_bass_kernel_spmd(nc, in_maps, list(range(N_CORES)))

    attn = np.concatenate([res.results[c]["attn"] for c in range(N_CORES)], axis=0)
    out = np.concatenate([res.results[c]["out"] for c in range(N_CORES)], axis=0)
    out = out.reshape(B, H, T, Sdim, Wdim, Cdim)
    attn = attn.reshape(NG * Wdim, Sdim, Sdim)
    return out, attn


# revision 18
# speedup vs baseline: 1.3482x; 1.3482x over previous
"""AxialAttention Trainium2 kernel (8-core SPMD, batch-parallel).

Problem: q,k,v [2,8,4,512,8,64] f32, attention over axis 3 (H=512).
Flattened: 512 independent attentions of [S=512, c=64], shared masks.
Outputs: out (same shape as q) and attn [512, 512, 512] probabilities.

Sharding: 64 (b,h,t) groups x 8 w-batches; core i owns groups [8i, 8i+8).
Host pre-transposes q,k into [g, wpair, (2w,c)=128, S] fp16 tiles and
pre-casts v to fp16, so the device does no q/k/v preprocessing.

Per-core pipeline per batch (w):
  S(psum)[ic] = ident16 @ maskbias16  +  qT16.T @ kT16     (fp16 PE)
  M, denom    = exp(S/8) with accum_out (ACT; mask folded in additively)
  r           = 1/denom                (DVE reciprocal)
  A           = M * head_mask          (DVE tensor_tensor f32)
  attn        = A * r                  (DVE tensor_scalar)  -> DMA out
  a16         = fp16(attn)             (DVE cast)
  a16^T       = PE transposes (fp16) -> psum -> evict fp16 (ACT/DVE)
  out[ic]     = sum_jc a16^T[jc,ic].T @ v16[jc]  (fp16 PE) -> copy -> DMA
"""
import sys
import numpy as np

sys.path.insert(0, "/opt/trn_rl_repo")

import concourse.bacc as bacc
import concourse.tile as tile
from concourse import mybir
from concourse.bass_utils import run_bass_kernel_spmd
from concourse.masks import make_identity

F32 = mybir.dt.float32
F16 = mybir.dt.float16
EXP = mybir.ActivationFunctionType.Exp
COPY = mybir.ActivationFunctionType.Copy
MUL = mybir.AluOpType.mult
BYP = mybir.AluOpType.bypass

N_CORES = 8
G_PER_CORE = 8   # (b,h,t) groups per core
W = 8            # w batches per group
S = 512          # sequence (H axis)
C = 64           # head dim
IC = S // 128    # i-chunks per batch
MASK_NEG = -49152.0  # exactly representable in fp16; exp((S+this)/8) == 0


def build_nc():
    nc = bacc.Bacc("TRN2", target_bir_lowering=False, debug=False,
                   num_devices=N_CORES)

    qT_ext = nc.dram_tensor("qT", [G_PER_CORE, 4, 128, S], F16, kind="ExternalInput")
    kT_ext = nc.dram_tensor("kT", [G_PER_CORE, 4, 128, S], F16, kind="ExternalInput")
    v_ext = nc.dram_tensor("v", [G_PER_CORE, S, W * C], F16, kind="ExternalInput")
    mb_ext = nc.dram_tensor("maskbias", [S, S], F16, kind="ExternalInput")
    hm_ext = nc.dram_tensor("headmask", [S, S], F32, kind="ExternalInput")
    attn_ext = nc.dram_tensor("attn", [G_PER_CORE * W, S, S], F32, kind="ExternalOutput")
    out_ext = nc.dram_tensor("out", [G_PER_CORE, S, W * C], F32, kind="ExternalOutput")

    with tile.TileContext(nc) as tc:
        with (
            tc.tile_pool(name="const", bufs=1) as const,
            tc.tile_pool(name="slab", bufs=12) as slab,
            tc.tile_pool(name="trans", bufs=8) as trans,
            tc.tile_pool(name="soft", bufs=8) as soft,
            tc.tile_pool(name="att", bufs=14) as att,
            tc.tile_pool(name="small", bufs=40) as small,
            tc.tile_pool(name="oslab", bufs=2) as oslab,
            tc.tile_pool(name="ps_s", bufs=4, space="PSUM") as ps_s,
            tc.tile_pool(name="ps_ta", bufs=1, space="PSUM") as ps_ta,
            tc.tile_pool(name="ps_o", bufs=2, space="PSUM") as ps_o,
        ):
            identf = const.tile([128, 128], F32)
            make_identity(nc, identf)
            ident16 = const.tile([128, 128], F16)
            nc.vector.tensor_copy(ident16, identf)

            # masks (shared across all batches), as 4 i-chunk tiles [128, S]
            mb16 = []
            hm_t = []
            for ic in range(IC):
                t = const.tile([128, S], F16, tag=f"mb{ic}")
                nc.sync.dma_start(out=t, in_=mb_ext[ic * 128:(ic + 1) * 128, :])
                mb16.append(t)
                th = const.tile([128, S], F16, tag=f"hm{ic}")
                nc.gpsimd.dma_start(out=th, in_=hm_ext[ic * 128:(ic + 1) * 128, :])
                hm_t.append(th)

            for g in range(G_PER_CORE):
                # ---- load qT/kT (fp16, pre-transposed on host) and v16 ----
                qT, kT, v16 = [], [], []
                for wp in range(4):
                    qt = slab.tile([128, S], F16, tag="qT")
                    nc.sync.dma_start(out=qt, in_=qT_ext[g, wp])
                    qT.append(qt)
                    kt = slab.tile([128, S], F16, tag="kT")
                    nc.sync.dma_start(out=kt, in_=kT_ext[g, wp])
                    kT.append(kt)
                for hc in range(IC):
                    v6 = slab.tile([128, W * C], F16, tag="v16")
                    nc.sync.dma_start(out=v6, in_=v_ext[g, hc * 128:(hc + 1) * 128, :])
                    v16.append(v6)

                o_slab = [oslab.tile([128, W * C], F32, tag=f"os{ic}", name=f"oslab{ic}")
                          for ic in range(IC)]

                for w in range(W):
                    wp, half = w // 2, (w % 2) * 64
                    b = g * W + w

                    sps = []
                    for ic in range(IC):
                        sp = ps_s.tile([128, S], F32, tag="S")
                        nc.tensor.matmul(sp, lhsT=ident16, rhs=mb16[ic],
                                         start=True, stop=False)
                        sps.append(sp)
                    for ic in range(IC):
                        nc.tensor.matmul(
                            sps[ic],
                            lhsT=qT[wp][half:half + 64, ic * 128:(ic + 1) * 128],
                            rhs=kT[wp][half:half + 64, :],
                            start=False, stop=True,
                        )
                    ta_banks = [ps_ta.tile([128, 2 * S], F16, tag=f"ta{i}", name=f"ta{i}")
                                for i in range(2)]
                    m_ts = []
                    den = small.tile([128, IC], F32, tag="den")
                    for ic in range(IC):
                        m_t = soft.tile([128, S], F16, tag="M")
                        nc.scalar.activation(m_t, sps[ic], EXP, scale=0.125,
                                             accum_out=den[:, ic:ic + 1])
                        m_ts.append(m_t)
                    r_t = small.tile([128, IC], F32, tag="r")
                    nc.vector.reciprocal(r_t, den)
                    for ic in range(IC):
                        m_t = m_ts[ic]
                        a_t = soft.tile([128, S], F16, tag="A")
                        nc.vector.tensor_tensor(out=a_t, in0=m_t, in1=hm_t[ic],
                                                op=MUL)
                        a16 = soft.tile([128, S], F16, tag="A16")
                        nc.vector.tensor_scalar(out=a16, in0=a_t,
                                                scalar1=r_t[:, ic:ic + 1],
                                                op0=MUL, scalar2=None, op1=BYP)
                        at_t = att.tile([128, S], F32, tag="attn")
                        nc.vector.tensor_copy(at_t, a16)
                        nc.sync.dma_start(
                            out=attn_ext[b, ic * 128:(ic + 1) * 128, :],
                            in_=at_t,
                        )
                        for jc in range(IC):
                            nc.tensor.transpose(
                                ta_banks[jc // 2][:, (jc % 2) * S + ic * 128:
                                                  (jc % 2) * S + (ic + 1) * 128],
                                a16[:, jc * 128:(jc + 1) * 128],
                                ident16,
                            )
                    atT = []
                    for jc in range(IC):
                        t = trans.tile([128, S], F16, tag=f"atT{jc}", bufs=6)
                        src = ta_banks[jc // 2][:, (jc % 2) * S:(jc % 2 + 1) * S]
                        if jc <= 1:
                            nc.vector.tensor_copy(t, src)
                        else:
                            nc.scalar.activation(t, src, COPY)
                        atT.append(t)

                    for ic in range(IC):
                        op = ps_o.tile([128, C], F32, tag="o")
                        for jc in range(IC):
                            nc.tensor.matmul(
                                op,
                                lhsT=atT[jc][:, ic * 128:(ic + 1) * 128],
                                rhs=v16[jc][:, w * C:(w + 1) * C],
                                start=(jc == 0), stop=(jc == IC - 1),
                            )
                        nc.vector.tensor_copy(
                            o_slab[ic][:, w * C:(w + 1) * C], op)

                for ic in range(IC):
                    nc.sync.dma_start(
                        out=out_ext[g, ic * 128:(ic + 1) * 128, :],
                        in_=o_slab[ic],
                    )

    nc.compile()
    return nc


_NC_CACHE = None


def make_in_maps(q, k, v, attention_mask, head_mask):
    q = np.ascontiguousarray(q, dtype=np.float32)
    k = np.ascontiguousarray(k, dtype=np.float32)
    v = np.ascontiguousarray(v, dtype=np.float32)
    B, H, T, Sdim, Wdim, Cdim = q.shape  # [2,8,4,512,8,64]
    NG = B * H * T                        # 64 groups

    # [NG, S, W, C] -> [NG, wp=4, (wodd,c)=128, S] fp16
    def to_T(x):
        xr = x.reshape(NG, Sdim, 4, 2, Cdim)
        return np.ascontiguousarray(
            xr.transpose(0, 2, 3, 4, 1).reshape(NG, 4, 2 * Cdim, Sdim)
        ).astype(np.float16)

    qT = to_T(q)
    kT = to_T(k)
    v16 = v.reshape(NG, Sdim, Wdim * Cdim).astype(np.float16)

    mask = np.asarray(attention_mask).reshape(Sdim, Sdim)
    maskbias = (((mask == 0).astype(np.float32)) * np.float32(MASK_NEG)).astype(np.float16)
    hm = np.ascontiguousarray(
        np.asarray(head_mask).reshape(Sdim, Sdim), dtype=np.float32)

    in_maps = []
    for c in range(N_CORES):
        sl = slice(c * G_PER_CORE, (c + 1) * G_PER_CORE)
        in_maps.append({
            "qT": np.ascontiguousarray(qT[sl]),
            "kT": np.ascontiguousarray(kT[sl]),
            "v": np.ascontiguousarray(v16[sl]),
            "maskbias": maskbias,
            "headmask": hm,
        })
    return in_maps, (B, H, T, Sdim, Wdim, Cdim)


def kernel(q, k, v, attention_mask, head_mask):
    global _NC_CACHE
    if _NC_CACHE is None:
        _NC_CACHE = build_nc()
    nc = _NC_CACHE

    in_maps, (B, H, T, Sdim, Wdim, Cdim) = make_in_maps(
        q, k, v, attention_mask, head_mask)
    NG = B * H * T

    res = run_bass_kernel_spmd(nc, in_maps, list(range(N_CORES)))

    attn = np.concatenate([res.results[c]["attn"] for c in range(N_CORES)], axis=0)
    out = np.concatenate([res.results[c]["out"] for c in range(N_CORES)], axis=0)
    out = out.reshape(B, H, T, Sdim, Wdim, Cdim)
    attn = attn.reshape(NG * Wdim, Sdim, Sdim)
    return out, attn
